# revision 46
# baseline (speedup 1.0000x reference)
"""Trainium2 Bass kernel for a GNN message-passing layer.

Reference computation (per graph):
    src,dst = edge_indices
    h   = gelu(concat(x[src], x[dst], e) @ W1m + b1m)          # [E, H]
    msg = h @ W2m + b2m                                        # [E, H]
    agg = segment_sum(msg, dst)                                # [N, H]
    u   = gelu(concat(x, agg) @ W1u + b1u)                     # [N, H]
    out = u @ W2u + b2u                                        # [N, D]

Device strategy (8 cores = 2 graphs x 4 node partitions):
  - By linearity, W2m is applied AFTER aggregation: agg = segsum(h) @ W2m + deg*b2m.
  - The src-side projection Psrc = x@W1m[:D] is computed once per node
    (stage 1), stored bf16 in DRAM, and row-gathered per edge for most
    chunks; a tunable subset of chunks instead receives x[src]
    feature-major from the host and projects on the PE (hybrid src path
    balancing the gather's Pool-engine cost against PE streams).
  - The dst-side term x[dst]@W1m[D:2D] and edge term e@W1m[2D:] + b1m are
    dense per-edge matmuls accumulated in PSUM (host supplies x[dst] and
    e feature-major in edge-slot order).
  - h = gelu(Psrc[src] + PSUM) token-major; for matmul-sourced chunks the
    gelu reads the fully-accumulated PSUM directly (no vector add).
  - Scatter-add via one-hot matmul: sel[e,n] = (dst_rel[e]==n); per node
    group aggT[h,n] += h_chunk.T @ sel accumulated in PSUM.  Scatter
    emission is software-pipelined 2 groups behind the projection
    matmuls so the PE never stalls waiting on the add/gelu chain.
  - Host bin-packs nodes into 128-node groups with balanced edge counts
    so every group needs the same k_blk chunks on every core.
  - Node-update MLP (stage 3) is interleaved after each 512-node group.
"""

import sys

sys.path.insert(0, "/opt/trn_rl_repo")

import heapq

import numpy as np
import ml_dtypes

import concourse.bacc as bacc
import concourse.mybir as mybir
import concourse.tile as tile
from concourse.bass_utils import run_bass_kernel_spmd

BF16 = ml_dtypes.bfloat16

B, N, E = 2, 10000, 160000
D, F, H = 128, 64, 256
NCORES = 8
CPG = NCORES // B          # cores per graph = 4
NBLK = 20                  # node groups per core
BLK = 128                  # nodes per group
NSLICE = NBLK * BLK        # 2560 nodes per core
NPAD = CPG * NSLICE        # 10240 padded nodes per graph
NG5 = NSLICE // 512        # 512-node stage-3 groups per core

f32 = mybir.dt.float32
bf16 = mybir.dt.bfloat16
i16 = mybir.dt.int16

GG = 8        # chunks per dma_gather (1024 indices, the HW max)
GCH = 4       # chunks per compute group (PSUM-bank limited)
LOOKAHEAD = 2  # groups of scatter deferral (PE software pipeline)

_BUILD_CACHE = {}


def _schedule(k_blk):
    """Per-block chunk sourcing. Returns (mm_chunks[blk], ngr).
    mm_chunks[blk] = number of TRAILING chunks in the block that are
    matmul-sourced (host xsrc); the leading chunks are gather-sourced.
    Blocks 0-3 are fully matmul-sourced so the edge pipeline can start
    before the Psrc gather table is written and the first gathers land."""
    ngr = (k_blk + GCH - 1) // GCH
    mm_chunks = []
    for blk in range(NBLK):
        if blk < 4:
            mm_chunks.append(k_blk)
        else:
            mm_chunks.append(k_blk - (ngr - 1) * GCH)  # last group only
    return mm_chunks, ngr


def _build(k_blk):
    nchunk = NBLK * k_blk
    ecap = nchunk * 128
    mm_chunks, ngr = _schedule(k_blk)
    # xsrc column offset per block (packed block-major, mm chunks only)
    xoff = np.concatenate([[0], np.cumsum(mm_chunks)]).astype(int)
    xcap = int(xoff[-1]) * 128

    nc = bacc.Bacc(None, num_swdge_queues=4)

    # ---- external inputs (per-core) ----
    nft = nc.dram_tensor("nft", [D, NPAD], bf16, kind="ExternalInput")
    wsrc0 = nc.dram_tensor("wsrc0", [D, H], bf16, kind="ExternalInput")
    nfs = nc.dram_tensor("nfs", [D, NSLICE], bf16, kind="ExternalInput")
    eft = nc.dram_tensor("eft", [F + 1, ecap], bf16, kind="ExternalInput")
    xdst = nc.dram_tensor("xdst", [D, ecap], bf16, kind="ExternalInput")
    xsrc = nc.dram_tensor("xsrc", [D, max(xcap, 128)], bf16, kind="ExternalInput")
    gidx = nc.dram_tensor("gidx", [128, ecap // 16], i16, kind="ExternalInput")
    dstrel = nc.dram_tensor("dstrel", [128, nchunk], bf16, kind="ExternalInput")
    degrow = nc.dram_tensor("degrow", [1, NSLICE], bf16, kind="ExternalInput")
    iotaw = nc.dram_tensor("iotaw", [128, k_blk * 128], bf16, kind="ExternalInput")
    # all small bf16 weights packed into one tensor (one DMA):
    # cols: w1e 256 | wsrc 256 | wdst 256 | w2m 512 | w1u 768 | w2u 256 |
    #       b2mr 256 | b2ur 128 | onesr 128 | iotat 128
    WPACK_COLS = 256 + 256 + 256 + 512 + 768 + 256 + 256 + 128 + 128 + 128
    wpack = nc.dram_tensor("wpack", [128, WPACK_COLS], bf16, kind="ExternalInput")
    b1uc = nc.dram_tensor("b1uc", [128, 2], f32, kind="ExternalInput")

    out = nc.dram_tensor("out", [NSLICE, D], f32, kind="ExternalOutput")

    PSB = 4       # stage-1 blocks per PSUM tile / cast / DRAM write
    BPG = k_blk * 128          # slot columns per block
    G5C = 4 * BPG              # slot columns per g5

    with tile.TileContext(nc) as tc:
        with (
            tc.tile_pool(name="const", bufs=1) as cpool,
            tc.tile_pool(name="dram", bufs=1, space="DRAM") as dpool,
            tc.tile_pool(name="eftp", bufs=3) as eftp,
            tc.tile_pool(name="xdp", bufs=3) as xdp,
            tc.tile_pool(name="xsp", bufs=3) as xsp,
            tc.tile_pool(name="gath", bufs=6) as gpool,
            tc.tile_pool(name="sel", bufs=2) as selp,
            tc.tile_pool(name="hwork", bufs=4) as hp,
            tc.tile_pool(name="stg1", bufs=6) as stgp,
            tc.tile_pool(name="nftp", bufs=4) as nftp,
            tc.tile_pool(name="aggp", bufs=2) as aggp,
            tc.tile_pool(name="cp", bufs=2) as cp,
            tc.tile_pool(name="s3", bufs=2) as s3p,
            tc.tile_pool(name="psA", bufs=3, space="PSUM") as psA,
            tc.tile_pool(name="agg", bufs=2, space="PSUM") as psG,
        ):
            def load(dram_t, shape, dtype):
                t = cpool.tile(shape, dtype, tag=dram_t.name)
                nc.sync.dma_start(out=t[:], in_=dram_t[:])
                return t

            # sync queue: stage-1 dependencies first (a tiny wsrc copy and
            # the first nft stripes), then the Psrc table writes, then
            # everything the later phases need.
            wsrc_s = load(wsrc0, [D, H], bf16)
            NSTRIPE = NPAD // (PSB * 128)
            nft_s = [None] * NSTRIPE

            def load_stripe(b):
                t = nftp.tile([D, PSB * 128], bf16, tag="nftl",
                              name=f"nftl{b}")
                nc.sync.dma_start(
                    out=t[:], in_=nft[:, b * PSB * 128:(b + 1) * PSB * 128])
                nft_s[b] = t

            load_stripe(0)
            load_stripe(1)
            load_stripe(2)
            wpack_s = load(wpack, [128, WPACK_COLS], bf16)
            dstrel_s = load(dstrel, [128, nchunk], bf16)
            b1uc_s = load(b1uc, [128, 2], f32)
            iotaw_s = load(iotaw, [128, k_blk * 128], bf16)

            co = np.concatenate([[0], np.cumsum([256, 256, 256, 512, 768, 256,
                                                 256, 128, 128, 128])]).astype(int)
            w1e_s = wpack_s[0:F + 1, co[0]:co[1]]
            wdst_s = wpack_s[:, co[2]:co[3]]
            w2m_s = wpack_s[:, co[3]:co[4]]
            w1u_s = wpack_s[:, co[4]:co[5]]
            w2u_s = wpack_s[:, co[5]:co[6]]
            b2mr_s = wpack_s[0:1, co[6]:co[7]]
            b2ur_s = wpack_s[0:1, co[7]:co[8]]
            onesr_s = wpack_s[0:1, co[8]:co[9]]

            # edge-operand stream bundles (block ranges).  The early ones go
            # out on the scalar engine's DMA queue so they don't wait behind
            # the sync queue (which carries nft + the Psrc table writes).
            ET = [None] * NBLK   # per-block (tile, col offset)
            XT = [None] * NBLK
            XS = [None] * NBLK

            def issue_bundle(b0, eng):
                nb = 2
                c0 = b0 * BPG
                w = nb * BPG
                et = eftp.tile([F + 1, 2 * BPG], bf16, tag="eft",
                               name=f"et_{b0}")
                eng.dma_start(out=et[:, 0:w], in_=eft[:, c0:c0 + w])
                xt = xdp.tile([128, 2 * BPG], bf16, tag="xd", name=f"xt_{b0}")
                eng.dma_start(out=xt[:, 0:w], in_=xdst[:, c0:c0 + w])
                x0, x1 = int(xoff[b0]) * 128, int(xoff[b0 + nb]) * 128
                xs = None
                if x1 > x0:
                    xs = xsp.tile([128, 2 * BPG], bf16, tag="xs",
                                  name=f"xs_{b0}")
                    eng.dma_start(out=xs[:, 0:x1 - x0], in_=xsrc[:, x0:x1])
                for j in range(nb):
                    ET[b0 + j] = (et, j * BPG)
                    XT[b0 + j] = (xt, j * BPG)
                    XS[b0 + j] = (xs, (int(xoff[b0 + j]) - int(xoff[b0])) * 128)

            issue_bundle(0, nc.scalar)
            issue_bundle(2, nc.scalar)

            # Psrc table in DRAM (gather source)
            pall = dpool.tile([NPAD, H], bf16)

            # selection matrices for blocks 0-1 emitted ahead of the
            # stage-1 casts so they're ready before the first scatters
            selB_early = {}
            for blk in range(2):
                selB = selp.tile([128, k_blk, 128], bf16, tag="sel",
                                 name=f"selB{blk}")
                nc.vector.tensor_tensor(
                    out=selB[:],
                    in0=dstrel_s[:, blk * k_blk:(blk + 1) * k_blk]
                        .to_broadcast([128, k_blk, 128]),
                    in1=iotaw_s.rearrange("p (a n) -> p a n", n=128),
                    op=mybir.AluOpType.is_equal,
                )
                selB_early[blk] = selB

            # ---- stage 1: Psrc projection table ----
            # Casts AND table writes alternate between the Vector+sync and
            # Scalar queues so no single ring paces the pipeline.
            cast_engs = [nc.vector.tensor_copy, nc.scalar.copy]
            write_engs = [nc.sync, nc.scalar]
            for sb in range(NSTRIPE):
                nb0 = sb * PSB
                t = nft_s[sb]
                if sb + 3 < NSTRIPE:
                    load_stripe(sb + 3)
                stg = stgp.tile([128, PSB, H], bf16, tag="stg")
                ps = psA.tile([128, PSB * H], f32, tag="psA")
                for j in range(PSB):
                    nc.tensor.matmul(
                        out=ps[:, j * H:(j + 1) * H],
                        lhsT=t[:, j * 128:(j + 1) * 128],
                        rhs=wsrc_s[:], start=True, stop=True,
                    )
                cast_engs[sb % 2](
                    out=stg[:].rearrange("p a h -> p (a h)"), in_=ps[:])
                write_engs[sb % 2].dma_start(
                    out=pall[nb0 * 128:(nb0 + PSB) * 128, :]
                        .rearrange("(a p) h -> p a h", p=128),
                    in_=stg[:],
                )

            # post-pall sync loads: gather indices first, then stage-3 needs,
            # then the remaining edge-operand bundles
            gidx_s = load(gidx, [128, ecap // 16], i16)
            issue_bundle(4, nc.sync)
            issue_bundle(6, nc.sync)
            nfs_s = load(nfs, [D, NSLICE], bf16)
            degrow_s = load(degrow, [1, NSLICE], bf16)
            for b0 in range(8, NBLK, 2):
                issue_bundle(b0, nc.sync)

            # per-512-node-group aggregates (feature-major, bf16), pooled:
            # only the filling g5 and the stage-3-consuming g5 are alive.
            aggT = {}

            # ---- stage 2 + interleaved stage 3, software-pipelined ----
            qn = 0
            blk_state = {}   # blk -> (ag0, ag1, selB)
            pend = []        # deferred scatter groups: (blk, g0, gw, h4, hoff)

            def emit_scatter(blk, g0, gw, h4, hoff):
                ag0, ag1, selB = blk_state[blk]
                for k in range(gw):
                    ck = g0 + k
                    hsl = hoff + k * H
                    nc.tensor.matmul(
                        out=ag0[:], lhsT=h4[:, hsl:hsl + 128],
                        rhs=selB[:, ck, :],
                        start=(ck == 0), stop=(ck == k_blk - 1),
                    )
                    nc.tensor.matmul(
                        out=ag1[:], lhsT=h4[:, hsl + 128:hsl + H],
                        rhs=selB[:, ck, :],
                        start=(ck == 0), stop=(ck == k_blk - 1),
                    )
                if g0 + gw == k_blk:
                    g5, j5 = blk // 4, blk % 4
                    csl = slice(j5 * 128, (j5 + 1) * 128)
                    nc.vector.tensor_copy(out=aggT[g5][0][:, csl], in_=ag0[:])
                    nc.scalar.copy(out=aggT[g5][1][:, csl], in_=ag1[:])
                    del blk_state[blk]
                    if j5 == 3:
                        emit_stage3(g5)

            def emit_stage3(g5):
                sl = slice(g5 * 512, (g5 + 1) * 512)
                agfT = [s3p.tile([128, 512], bf16, tag=f"agf{o}",
                                 name=f"agf{o}_{g5}") for o in range(2)]
                for o in range(2):
                    osl = slice(o * 128, (o + 1) * 128)
                    pa = psA.tile([128, 512], f32, tag="psA")
                    nc.tensor.matmul(out=pa[:], lhsT=w2m_s[:, o * 128:(o + 1) * 128],
                                     rhs=aggT[g5][0][:], start=True, stop=False)
                    nc.tensor.matmul(out=pa[:], lhsT=w2m_s[:, H + o * 128:H + (o + 1) * 128],
                                     rhs=aggT[g5][1][:], start=False, stop=False)
                    nc.tensor.matmul(out=pa[:], lhsT=b2mr_s[:, osl],
                                     rhs=degrow_s[:, sl], start=False, stop=True)
                    nc.vector.tensor_copy(out=agfT[o][:], in_=pa[:])
                uT = [s3p.tile([128, 512], bf16, tag=f"u{o}",
                               name=f"u{o}_{g5}") for o in range(2)]
                for o in range(2):
                    pu = psA.tile([128, 512], f32, tag="psA")
                    nc.tensor.matmul(out=pu[:], lhsT=w1u_s[:, 0 * H + o * 128:0 * H + (o + 1) * 128],
                                     rhs=nfs_s[:, sl], start=True, stop=False)
                    nc.tensor.matmul(out=pu[:], lhsT=w1u_s[:, 1 * H + o * 128:1 * H + (o + 1) * 128],
                                     rhs=agfT[0][:], start=False, stop=False)
                    nc.tensor.matmul(out=pu[:], lhsT=w1u_s[:, 2 * H + o * 128:2 * H + (o + 1) * 128],
                                     rhs=agfT[1][:], start=False, stop=True)
                    nc.scalar.activation(
                        out=uT[o][:], in_=pu[:],
                        func=mybir.ActivationFunctionType.Gelu_apprx_tanh,
                        bias=b1uc_s[:, o:o + 1],
                    )
                oc = cp.tile([128, 4, D], f32, tag="ocp")
                for j5 in range(4):
                    jsl = slice(j5 * 128, (j5 + 1) * 128)
                    po = psA.tile([128, 128], f32, tag="psA")
                    nc.tensor.matmul(out=po[:], lhsT=uT[0][:, jsl], rhs=w2u_s[:, 0:D],
                                     start=True, stop=False)
                    nc.tensor.matmul(out=po[:], lhsT=uT[1][:, jsl], rhs=w2u_s[:, D:2 * D],
                                     start=False, stop=False)
                    nc.tensor.matmul(out=po[:], lhsT=onesr_s[:], rhs=b2ur_s[:],
                                     start=False, stop=True)
                    nc.vector.tensor_copy(out=oc[:, j5, :], in_=po[:])
                nc.scalar.dma_start(
                    out=out[g5 * 512:(g5 + 1) * 512, :]
                        .rearrange("(a p) h -> p a h", p=128),
                    in_=oc[:],
                )

            for blk in range(NBLK):
                g5, j5 = blk // 4, blk % 4
                if j5 == 0:
                    aggT[g5] = [aggp.tile([128, 512], bf16, tag=f"aggT{o}",
                                          name=f"aggT{o}_{g5}") for o in range(2)]
                c00 = blk * k_blk
                n_gather = k_blk - mm_chunks[blk]
                # gathers for this block's leading chunks
                gts = []
                for g0 in range(0, n_gather, GG):
                    g1 = min(g0 + GG, n_gather)
                    gw = g1 - g0
                    gt = gpool.tile([128, GG, H], bf16, tag="gath")
                    nc.gpsimd.dma_gather(
                        gt[:, 0:gw, :],
                        pall[:],
                        gidx_s[:, (c00 + g0) * 8:(c00 + g0) * 8 + gw * 8],
                        num_idxs=gw * 128,
                        num_idxs_reg=gw * 128,
                        elem_size=H,
                        queue_num=qn,
                    )
                    qn = (qn + 1) % 4
                    gts.append(gt)
                # selection matrices for the whole block (one DVE op)
                if blk in selB_early:
                    selB = selB_early[blk]
                else:
                    selB = selp.tile([128, k_blk, 128], bf16, tag="sel")
                    nc.vector.tensor_tensor(
                        out=selB[:],
                        in0=dstrel_s[:, c00:c00 + k_blk]
                            .to_broadcast([128, k_blk, 128]),
                        in1=iotaw_s.rearrange("p (a n) -> p a n", n=128),
                        op=mybir.AluOpType.is_equal,
                    )
                ag0 = psG.tile([128, 128], f32, tag="agg")
                ag1 = psG.tile([128, 128], f32, tag="agg")
                blk_state[blk] = (ag0, ag1, selB)

                et, eo = ET[blk]
                xt, xo = XT[blk]
                xs, xso = XS[blk]
                for g0 in range(0, k_blk, GCH):
                    g1 = min(g0 + GCH, k_blk)
                    gw = g1 - g0
                    is_mm = g0 >= n_gather
                    pe4 = psA.tile([128, GCH * H], f32, tag="psA")
                    for k in range(gw):
                        ck = g0 + k
                        nc.tensor.matmul(
                            out=pe4[:, k * H:(k + 1) * H],
                            lhsT=et[:, eo + ck * 128:eo + (ck + 1) * 128],
                            rhs=w1e_s[:], start=True, stop=False,
                        )
                        if is_mm:
                            xcol = xso + (ck - (k_blk - mm_chunks[blk])) * 128
                            nc.tensor.matmul(
                                out=pe4[:, k * H:(k + 1) * H],
                                lhsT=xs[:, xcol:xcol + 128],
                                rhs=wsrc_s[:], start=False, stop=False,
                            )
                        nc.tensor.matmul(
                            out=pe4[:, k * H:(k + 1) * H],
                            lhsT=xt[:, eo + ck * 128:eo + (ck + 1) * 128],
                            rhs=wdst_s[:], start=False, stop=True,
                        )
                    h4 = hp.tile([128, GCH * H], bf16, tag="h")
                    if is_mm:
                        nc.scalar.activation(
                            out=h4[:, 0:gw * H], in_=pe4[:, 0:gw * H],
                            func=mybir.ActivationFunctionType.Gelu_apprx_tanh,
                        )
                    else:
                        gt = gts[g0 // GG]
                        gk = g0 % GG
                        s4 = hp.tile([128, GCH * H], bf16, tag="s")
                        nc.vector.tensor_add(
                            out=s4[:, 0:gw * H],
                            in0=gt[:, gk:gk + gw, :].rearrange("p a n -> p (a n)"),
                            in1=pe4[:, 0:gw * H],
                        )
                        nc.scalar.activation(
                            out=h4[:, 0:gw * H], in_=s4[:, 0:gw * H],
                            func=mybir.ActivationFunctionType.Gelu_apprx_tanh,
                        )
                    pend.append((blk, g0, gw, h4, 0))
                    if len(pend) > LOOKAHEAD:
                        emit_scatter(*pend.pop(0))
            while pend:
                emit_scatter(*pend.pop(0))

    nc.finalize()
    return nc


def _pack_nodes(deg):
    """Greedy LPT bin-packing: NPAD nodes -> 80 bins of exactly 128 nodes,
    balancing per-bin edge counts. Returns perm (group-major node order)."""
    nb = NPAD // BLK
    order = np.argsort(-deg, kind="stable")
    heap = [(0, b) for b in range(nb)]
    heapq.heapify(heap)
    cnt = np.zeros(nb, np.int64)
    load = np.zeros(nb, np.int64)
    bins = [[] for _ in range(nb)]
    for n in order:
        while True:
            l, b = heapq.heappop(heap)
            if cnt[b] < BLK:
                break
        bins[b].append(n)
        cnt[b] += 1
        load[b] += deg[n]
        if cnt[b] < BLK:
            heapq.heappush(heap, (load[b], b))
    perm = np.concatenate([np.asarray(b_, dtype=np.int64) for b_ in bins])
    return perm, load


def kernel(node_features, edge_indices, edge_features,
           W1m, b1m, W2m, b2m, W1u, b1u, W2u, b2u):
    node_features = np.asarray(node_features)
    edge_indices = np.asarray(edge_indices)
    edge_features = np.asarray(edge_features)

    nftg = np.zeros((B, D, NPAD), dtype=BF16)
    for g in range(B):
        nftg[g, :, :N] = np.asarray(node_features[g]).T.astype(BF16)

    perms, poss, k_blk = [], [], 1
    for g in range(B):
        deg = np.bincount(edge_indices[g, :, 1], minlength=NPAD)
        perm, load = _pack_nodes(deg)
        pos = np.empty(NPAD, np.int64)
        pos[perm] = np.arange(NPAD)
        perms.append(perm)
        poss.append(pos)
        k_blk = max(k_blk, int(np.ceil(load.max() / 128.0)))

    if k_blk not in _BUILD_CACHE:
        _BUILD_CACHE[k_blk] = _build(k_blk)
    nc = _BUILD_CACHE[k_blk]
    nchunk = NBLK * k_blk
    ecap = nchunk * 128
    mm_chunks, _ = _schedule(k_blk)
    xoff = np.concatenate([[0], np.cumsum(mm_chunks)]).astype(int)
    xcap = int(xoff[-1]) * 128

    iota = np.broadcast_to(np.arange(128, dtype=np.float32), (128, 128))
    wpack = np.zeros((128, 2944), dtype=BF16)
    co = np.concatenate([[0], np.cumsum([256, 256, 256, 512, 768, 256,
                                         256, 128, 128, 128])]).astype(int)
    wpack[0:F, co[0]:co[0] + 256] = np.asarray(W1m)[2 * D:].astype(BF16)
    wpack[F, co[0]:co[0] + 256] = np.asarray(b1m).astype(BF16)
    wpack[:, co[1]:co[2]] = np.asarray(W1m)[:D].astype(BF16)
    wpack[:, co[2]:co[3]] = np.asarray(W1m)[D:2 * D].astype(BF16)
    wpack[:, co[3]:co[4]] = np.asarray(W2m).reshape(2, 128, H).transpose(1, 0, 2).reshape(128, 2 * H).astype(BF16)
    wpack[:, co[4]:co[5]] = np.asarray(W1u).reshape(3, 128, H).transpose(1, 0, 2).reshape(128, 3 * H).astype(BF16)
    wpack[:, co[5]:co[6]] = np.asarray(W2u).reshape(2, 128, D).transpose(1, 0, 2).reshape(128, 2 * D).astype(BF16)
    wpack[0, co[6]:co[6] + 256] = np.asarray(b2m).astype(BF16)
    wpack[0, co[7]:co[7] + 128] = np.asarray(b2u).astype(BF16)
    wpack[0, co[8]:co[8] + 128] = BF16(1.0)
    wpack[:, co[9]:co[10]] = iota.astype(BF16)

    shared = {
        "wpack": wpack,
        "wsrc0": np.asarray(W1m)[:D].astype(BF16),
        "b1uc": np.asarray(b1u).reshape(2, 128).T.astype(np.float32).copy(),
        "iotaw": np.ascontiguousarray(np.broadcast_to(
            np.tile(np.arange(128, dtype=np.float32), k_blk),
            (128, k_blk * 128))).astype(BF16),
    }

    in_maps = []
    for c in range(NCORES):
        g, r = c // CPG, c % CPG
        src = edge_indices[g, :, 0].astype(np.int64)
        dst = edge_indices[g, :, 1].astype(np.int64)
        p = poss[g][dst]
        ecore = p // NSLICE
        eid = np.nonzero(ecore == r)[0]
        lp = p[eid] - r * NSLICE          # local position in [0, 2560)
        egroup = lp // BLK
        edrel = lp % BLK
        o = np.argsort(egroup, kind="stable")
        eid, egroup, edrel = eid[o], egroup[o], edrel[o]
        counts = np.bincount(egroup, minlength=NBLK)
        starts = np.concatenate([[0], np.cumsum(counts)])[:-1]
        within = np.arange(len(eid)) - starts[egroup]
        spos = egroup * (k_blk * 128) + within

        srcpad = np.zeros(ecap, dtype=np.int64)
        srcpad[spos] = src[eid]
        drel = np.full(ecap, -1.0, dtype=np.float64)
        drel[spos] = edrel

        eftc = np.zeros((F + 1, ecap), dtype=BF16)
        eftc[:F, spos] = edge_features[g, eid, :].T.astype(BF16)
        eftc[F, :] = BF16(1.0)
        xdstc = np.zeros((D, ecap), dtype=BF16)
        xdstc[:, spos] = nftg[g][:, dst[eid]]
        # xsrc for matmul-sourced (trailing) chunks, packed block-major
        xsrcc = np.zeros((D, max(xcap, 128)), dtype=BF16)
        for blk in range(NBLK):
            mc = mm_chunks[blk]
            if mc == 0:
                continue
            s0 = blk * k_blk * 128 + (k_blk - mc) * 128
            x0 = int(xoff[blk]) * 128
            seg = srcpad[s0:s0 + mc * 128]
            xsrcc[:, x0:x0 + mc * 128] = nftg[g][:, seg]

        gidxc = np.tile(srcpad.astype(np.int16).reshape(-1, 16).T, (8, 1))
        drelc = np.ascontiguousarray(drel.reshape(nchunk, 128).T).astype(BF16)
        degc = np.bincount(lp, minlength=NSLICE).astype(BF16)[None, :]
        perm_r = perms[g][r * NSLICE:(r + 1) * NSLICE]

        inp = dict(shared)
        inp["nft"] = nftg[g]
        inp["nfs"] = np.ascontiguousarray(nftg[g][:, perm_r])
        inp["eft"] = eftc
        inp["xdst"] = xdstc
        inp["xsrc"] = xsrcc
        inp["gidx"] = gidxc
        inp["dstrel"] = drelc
        inp["degrow"] = degc
        in_maps.append(inp)

    global _LAST_IN_MAPS
    _LAST_IN_MAPS = in_maps
    res = run_bass_kernel_spmd(nc, in_maps, core_ids=list(range(NCORES)))

    outp = np.zeros((B, NPAD, D), dtype=np.float32)
    for c in range(NCORES):
        g, r = c // CPG, c % CPG
        perm_r = perms[g][r * NSLICE:(r + 1) * NSLICE]
        outp[g, perm_r, :] = res.results[c]["out"]
    return outp[:, :N, :]
